# revision 34
# baseline (speedup 1.0000x reference)
"""Causal multi-head attention layer (train forward) on 8 Trainium2 NeuronCores.

Sharding: batch (4) x head-group (2 of 8 heads each) -> 8 cores.
Per core (batch b, head group g): project Q^T/K^T [512,S] and V [S,512] from
x_b in bf16 (fp32 PSUM accum), run causal attention head-pair-packed on the PE
array (row-tiled 64x128 score matmuls run both heads concurrently on the
row-split PE tiles), softmax rowsums ride a ones column on V (ctx matmul
M=65), reciprocal_approx_fast on DVE (partition-base 0 only!), and the
output projection with this core's Wo row block (bf16 out partials summed on
host). Per-pair staged normalization: h0 ctx stages at partitions 0-63 and
h1 at partitions 64-127 (cross-base PSUM copy), the per-head reciprocal rows
are DMA-broadcast into the matching partition halves, and the normalization
multiplies write ctx_w in place for BOTH halves -- no SBUF->SBUF DMA
roundtrip for the upper half. Triangle masking runs on GpSimd (SBUF-only
engine). Projection/output-projection matmuls are interleaved into the
exp-bound attention stream as deficit-paced fillers so the PE keeps busy
while ACT works through the exp stream; PE-output drains are emitted 2
work-units late and alternate DVE/ACT so neither in-order queue head-blocks.
"""
import numpy as np
import ml_dtypes

import concourse.bass as bass
import concourse.tile as tile
from concourse import bacc, mybir
from concourse.bass_utils import run_bass_kernel_spmd

F32 = mybir.dt.float32
BF16 = mybir.dt.bfloat16
AF = mybir.ActivationFunctionType
ALU = mybir.AluOpType

P = 128
D = 1024          # model dim
DC = 512          # per-core head dims (8 heads x 64)
HD = 64
NHC = 8           # heads per core
NPAIR = 4         # head pairs per core
FC = D // P       # 8 feature chunks
OC = DC // P      # 4 outdim chunks (= head pairs)
W = 512           # query window (fp32 PSUM bank)
WT = W // P       # token chunks per window
SCALE = 1.0 / 32.0  # 1/sqrt(D)

PROJ_COST = 3413.0   # ns: one 16-matmul projection group
OUTP_COST = 1707.0   # ns: one 8-matmul outproj token group


def build_nc(S=2048, num_devices=8, with_bv=False):
    NWIN = S // W

    nc = bacc.Bacc("TRN2", target_bir_lowering=False, debug=False,
                   num_devices=num_devices)
    xt = nc.dram_tensor("xt", [P, FC, S], BF16, kind="ExternalInput").ap()
    wq = nc.dram_tensor("wq", [P, FC, DC], BF16, kind="ExternalInput").ap()
    wk = nc.dram_tensor("wk", [P, FC, DC], BF16, kind="ExternalInput").ap()
    wv = nc.dram_tensor("wv", [P, FC, DC], BF16, kind="ExternalInput").ap()
    wo = nc.dram_tensor("wo", [P, OC, D], BF16, kind="ExternalInput").ap()
    bias3 = nc.dram_tensor("bias3", [P, 4 * OC], F32,
                           kind="ExternalInput").ap()
    tri = nc.dram_tensor("tri", [P, P], BF16, kind="ExternalInput").ap()
    out = nc.dram_tensor("out", [S, D], BF16,
                         kind="ExternalOutput").ap()

    with tile.TileContext(nc) as tc:
        with tc.tile_pool(name="const", bufs=1) as cst, \
             tc.tile_pool(name="stage", bufs=3) as stg, \
             tc.tile_pool(name="pt", bufs=5) as ptp, \
             tc.tile_pool(name="small", bufs=2) as sml, \
             tc.tile_pool(name="stgp", bufs=2) as stgp, \
             tc.tile_pool(name="psA", bufs=1, space="PSUM") as psA, \
             tc.tile_pool(name="psC", bufs=1, space="PSUM") as psC:

            mm_ctr = [0]

            def mm_tile():
                i = mm_ctr[0]
                mm_ctr[0] += 1
                return psA.tile([P, 1024], F32, tag=f"s{i % 3}",
                                name=f"mm_s{i % 3}")

            eng_ctr = [0]

            def nxt():
                eng_ctr[0] += 1
                return eng_ctr[0]

            def _copy(out_, in_, dve_only=False):
                if dve_only or nxt() % 2 == 0:
                    nc.vector.tensor_copy(out_, in_)
                else:
                    nc.scalar.copy(out_, in_)

            # --- constants (already bf16/pre-arranged from host) ---
            tri_bf = cst.tile([P, P], BF16, tag="tri")
            b3_sb = cst.tile([P, 4 * OC], F32, tag="bias3")
            bv_sb = b3_sb[0:HD, 2 * OC:2 * OC + NHC]
            w_sbs = {}
            for name in ("wq", "wk", "wv"):
                w_sbs[name] = cst.tile([P, FC, DC], BF16, tag=name, name=name)
            wo_sb = cst.tile([P, OC, D], BF16, tag="wo")

            # --- per-window tiles ---
            xT_w, qT_w, kT_w, v_w, ctx_w = [], [], [], [], []
            for j in range(NWIN):
                xT_w.append(cst.tile([P, FC, W], BF16, tag=f"xT{j}",
                                     name=f"xT{j}"))
                qT_w.append(cst.tile([P, OC, W], BF16, tag=f"qT{j}",
                                     name=f"qT{j}"))
                kT_w.append(cst.tile([P, OC, W], BF16, tag=f"kT{j}",
                                     name=f"kT{j}"))
                v_w.append(cst.tile([P, WT, NHC, HD + 1], BF16, tag=f"v{j}",
                                    name=f"v{j}"))
                ctx_w.append(cst.tile([P, NPAIR, W], BF16, tag=f"ctx{j}",
                                      name=f"ctx{j}"))
                nc.vector.memset(v_w[j][:, :, :, HD:HD + 1], 1.0)

            # --- startup DMAs; one shared DMA engine, so order by need.
            # Gating inputs (xT0/wq/wk first halves) are chunked per feature
            # chunk so the first projections' fc-accumulation tracks DMA
            # progress. All later bulk prefetch goes on the scalar ring so
            # the sync/gpsimd rings stay free for small latency-critical
            # DMAs (bc broadcasts, shw, out chunks). ---
            HC = DC // 2
            for fc in range(FC):
                nc.gpsimd.dma_start(w_sbs["wq"][:, fc, 0:HC],
                                    wq[:, fc, 0:HC])
                q = nc.sync if fc % 2 == 0 else nc.scalar
                q.dma_start(xT_w[0][:, fc, :], xt[:, fc, 0:W])
            nc.gpsimd.dma_start(b3_sb[:], bias3[:])
            nc.gpsimd.dma_start(tri_bf[:], tri[:])
            for fc in range(FC):
                q = nc.sync if fc % 2 == 0 else nc.gpsimd
                q.dma_start(w_sbs["wk"][:, fc, 0:HC], wk[:, fc, 0:HC])
            for fc in range(FC):
                q = nc.gpsimd if fc % 2 == 0 else nc.sync
                q.dma_start(w_sbs["wv"][:, fc, :], wv[:, fc, :])
            nc.scalar.dma_start(w_sbs["wq"][:, :, HC:DC], wq[:, :, HC:DC])
            nc.scalar.dma_start(w_sbs["wk"][:, :, HC:DC], wk[:, :, HC:DC])
            for jj in range(1, NWIN):
                nc.scalar.dma_start(xT_w[jj][:], xt[:, :, jj * W:(jj + 1) * W])
            nc.scalar.dma_start(wo_sb[:], wo[:])

            # --- projection / output-projection units (PE filler work) ---
            def proj_qk(j, wname, b_off, og):
                dst = qT_w[j] if wname == "wq" else kT_w[j]
                w_sb = w_sbs[wname]
                ps = mm_tile()
                for half in range(2):
                    oc = og * 2 + half
                    for fc in range(FC):
                        nc.tensor.matmul(
                            ps[:, half * W:(half + 1) * W],
                            w_sb[:, fc, oc * P:(oc + 1) * P],
                            xT_w[j][:, fc, :],
                            start=(fc == 0), stop=(fc == FC - 1))
                    hv = ps[:, half * W:(half + 1) * W]
                    b_col = b3_sb[:, b_off + oc:b_off + oc + 1]

                    def drain(hv=hv, b_col=b_col, oc=oc):
                        if nxt() % 2 == 0:
                            nc.vector.tensor_scalar(dst[:, oc, :], hv, b_col,
                                                    None, ALU.add)
                        else:
                            nc.scalar.activation(dst[:, oc, :], hv,
                                                 AF.Identity, bias=b_col)
                    if j == 0:
                        drain()     # window-0 ramp: attention waits on these
                    else:
                        defer(2, drain)

            def proj_v(j, tg):
                ps = mm_tile()
                for half in range(2):
                    t = tg * 2 + half
                    for fc in range(FC):
                        nc.tensor.matmul(
                            ps[:, half * W:(half + 1) * W],
                            xT_w[j][:, fc, t * P:(t + 1) * P],
                            w_sbs["wv"][:, fc, :],
                            start=(fc == 0), stop=(fc == FC - 1))
                dv = ps.rearrange("p (t h n) -> p t h n", t=2, h=NHC)

                def drain():
                    _copy(v_w[j][:, tg * 2:tg * 2 + 2, :, 0:HD], dv)
                if j == 0:
                    drain()
                else:
                    defer(2, drain)

            def outproj_unit(j, t, dve_drain):
                tokc = j * WT + t
                ps = mm_tile()
                # pr-major-first, last pair deferred to the end of BOTH psum
                # halves: the first six matmuls only need pairs 0-2, so at
                # the kernel tail they overlap the final pair's norm chain
                for nb in range(2):
                    for pr in range(NPAIR - 1):
                        nc.tensor.matmul(
                            ps[:, nb * W:(nb + 1) * W],
                            ctx_w[j][:, pr, t * P:(t + 1) * P],
                            wo_sb[:, pr, nb * 512:(nb + 1) * 512],
                            start=(pr == 0), stop=False)
                for nb in range(2):
                    nc.tensor.matmul(
                        ps[:, nb * W:(nb + 1) * W],
                        ctx_w[j][:, NPAIR - 1, t * P:(t + 1) * P],
                        wo_sb[:, NPAIR - 1, nb * 512:(nb + 1) * 512],
                        start=False, stop=True)
                def drain():
                    ost = stg.tile([P, D], BF16, tag="ostage", name="ost")
                    _copy(ost[:], ps[:], dve_only=dve_drain)
                    nc.sync.dma_start(out[tokc * P:(tokc + 1) * P, :], ost[:])
                defer(2, drain)

            # --- deferred drains: a drain emitted right after its producer
            # matmuls reaches the in-order DVE/ACT queue head while the PE
            # is still several units behind, and head-blocks the queue.
            # Defer by 2 PE work-units (< the 3-deep psum rotation, so the
            # drain is always emitted before the strip's next writer). ---
            defer_q = []

            def defer(delay, fn):
                defer_q.append([delay, fn])

            def tick(n=1):
                for it in defer_q[:]:
                    it[0] -= n
                    if it[0] <= 0:
                        defer_q.remove(it)
                        it[1]()

            # --- filler scheduler: PE work to hide ACT(exp) latency ---
            filler_q = []
            debt = [0.0]

            def pump(add_ns):
                debt[0] += add_ns
                while filler_q and debt[0] >= filler_q[0][0]:
                    cost, fn = filler_q.pop(0)
                    fn()
                    tick()
                    debt[0] -= cost
                if not filler_q and debt[0] > 4000.0:
                    debt[0] = 4000.0

            def flush():
                while filler_q:
                    _, fn = filler_q.pop(0)
                    fn()
                    tick()
                debt[0] = 0.0

            # --- attention for one (window, head-pair); returns norm closure ---
            def attn_pair(j, p):
                skc_hi = WT * (j + 1)
                last_win = (j == NWIN - 1)
                ctx0 = psC.tile([P, W], F32, tag="c0", name="ctx0")
                ctx1 = psC.tile([P, W], F32, tag="c1", name="ctx1")
                for skc in range(skc_hi):
                    jk, tk = divmod(skc, WT)
                    rel = skc * P - j * W
                    vs = max(rel, 0)
                    n = W - vs
                    sp = mm_tile()
                    spv = sp.rearrange("p (h n) -> p h n", h=2)
                    nc.tensor.matmul(sp[:, vs:W],
                                     kT_w[jk][0:HD, p, tk * P:(tk + 1) * P],
                                     qT_w[j][0:HD, p, vs:W],
                                     start=True, stop=True)
                    nc.tensor.matmul(sp[:, W + vs:2 * W],
                                     kT_w[jk][HD:P, p, tk * P:(tk + 1) * P],
                                     qT_w[j][HD:P, p, vs:W],
                                     start=True, stop=True)
                    pt = ptp.tile([P, 1024], BF16, tag="pt", name="pt")
                    ptv = pt.rearrange("p (h n) -> p h n", h=2)
                    nc.scalar.activation(ptv[:, :, vs:W], spv[:, :, vs:W],
                                         AF.Exp, scale=SCALE)
                    if rel >= 0:
                        nc.gpsimd.tensor_tensor(
                            ptv[:, :, rel:rel + P], ptv[:, :, rel:rel + P],
                            tri_bf[:, None, :].to_broadcast([P, 2, P]),
                            ALU.mult)
                    st0 = (skc == 0)
                    sp0 = (skc == skc_hi - 1)
                    nc.tensor.matmul(ctx0[0:HD + 1, vs:W],
                                     v_w[jk][:, tk, 2 * p, :],
                                     ptv[:, 0, vs:W], start=st0, stop=sp0)
                    nc.tensor.matmul(ctx1[0:HD + 1, vs:W],
                                     v_w[jk][:, tk, 2 * p + 1, :],
                                     ptv[:, 1, vs:W], start=st0, stop=sp0)
                    # ACT time minus PE time for this chunk -> filler budget
                    tick()
                    pump((2.0 * n + 352.0) / 1.2 - (3.0 * n / 2.4 + 120.0))

                # rowsums (ones column) + unnormalized ctx drains; h1 ctx
                # stages directly at partitions 64-127 (cross-base copy) so
                # the normalization can write ctx_w[64:128] in place -- no
                # shw DMA roundtrip at all.
                stg2 = stgp.tile([P, 2, W], BF16, tag=f"sg{p % 2}",
                                 name="stg2")
                rws = []
                for h, cp in ((0, ctx0), (1, ctx1)):
                    rw = sml.tile([1, W], F32, tag=f"rw{(2 * p + h) % 4}",
                                  name="rw")
                    rws.append(rw)
                    nc.vector.tensor_copy(rw[:], cp[HD:HD + 1, :])
                    dst2 = (stg2[0:HD, 0, :] if h == 0
                            else stg2[HD:P, 1, :])
                    _copy(dst2, cp[0:HD, :], dve_only=(last_win or h == 1))

                bcs = []

                def stage1():
                    # reciprocal + bf16 cast + broadcast-DMA issue, one pair
                    # boundary after the rowsums landed. bc halves land at
                    # the partitions their head's ctx occupies.
                    bc = sml.tile([P, W], BF16, tag=f"bc{p % 2}", name="bc")
                    bcs.append(bc)
                    for h in range(2):
                        i = 2 * p + h
                        rcf = sml.tile([1, W], F32, tag=f"rf{i % 2}",
                                       name="rcf")
                        nc.vector.reciprocal_approx_fast(rcf[:], rws[h][:])
                        rc1 = sml.tile([1, W], BF16, tag=f"rc{i % 2}",
                                       name="rc1")
                        nc.vector.tensor_copy(rc1[:], rcf[:])
                        if j == NWIN - 1 and p >= 2:
                            ring = nc.scalar if h == 0 else nc.sync
                        else:
                            ring = nc.gpsimd
                        ring.dma_start(
                            bc[h * HD:(h + 1) * HD, :],
                            rc1[0:1, None, :].to_broadcast([1, HD, W]))

                def stage2():
                    # normalization multiplies; two pair boundaries after the
                    # rowsums, so the bc broadcasts are ready
                    bc = bcs[0]
                    for h in range(2):
                        i = 2 * p + h
                        dst = (ctx_w[j][0:HD, p, :] if h == 0
                               else ctx_w[j][HD:P, p, :])
                        src2 = (stg2[0:HD, 0, :] if h == 0
                                else stg2[HD:P, 1, :])
                        nc.vector.tensor_tensor(
                            dst, src2, bc[h * HD:(h + 1) * HD, :], ALU.mult)
                        if with_bv:
                            nc.vector.tensor_scalar(
                                dst, dst, bv_sb[:, i:i + 1], None, ALU.add)
                return stage1, stage2

            # --- main schedule ---
            proj_qk(0, "wq", 0, 0)
            proj_qk(0, "wk", OC, 0)
            proj_v(0, 0)
            proj_v(0, 1)

            deferred_outproj = []
            norm_q = []       # [age, pj, s1, s2, s1_done]
            norms_done = {}
            in_last_window = [False]

            def advance_norms():
                for ent in norm_q[:]:
                    ent[0] += 1
                    age, pj, f1, f2, s1_done = ent
                    if age >= 1 and not s1_done:
                        f1()
                        ent[4] = True
                    # late windows have long pairs: 2 boundaries of slack
                    # suffice for the bc broadcast, and the earlier finish
                    # lets outproj(2) interleave instead of piling at the tail
                    if age >= (3 if pj <= 1 else 2):
                        f2()
                        norm_q.remove(ent)
                        norms_done[pj] = norms_done.get(pj, 0) + 1
                        if norms_done[pj] == NPAIR and pj < NWIN - 1:
                            units = [(OUTP_COST,
                                      lambda jj=pj, tt=t:
                                      outproj_unit(jj, tt, True))
                                     for t in range(WT)]
                            if in_last_window[0]:
                                filler_q.extend(units)
                            else:
                                deferred_outproj.extend(units)

            def proj_units(jn):
                return [
                    (PROJ_COST, lambda: proj_qk(jn, "wq", 0, 0)),
                    (PROJ_COST, lambda: proj_qk(jn, "wk", OC, 0)),
                    (PROJ_COST, lambda: proj_v(jn, 0)),
                    (PROJ_COST, lambda: proj_v(jn, 1)),
                    (PROJ_COST, lambda: proj_qk(jn, "wq", 0, 1)),
                    (PROJ_COST, lambda: proj_qk(jn, "wk", OC, 1)),
                ]

            # windows 0,1 sequential; windows 2,3 pair-interleaved so
            # window 3's exp load spreads into the PE-rich window-2 phase.
            # proj(3) runs as fillers inside the mixed phase (needed only
            # by the first (3,*) pair).
            sched = [(0, p) for p in range(NPAIR)]
            sched += [(1, p) for p in range(NPAIR)]
            sched += [(2, 0), (2, 1), (2, 2), (3, 0),
                      (2, 3), (3, 1), (3, 2), (3, 3)]

            for (j, p) in sched:
                if (j, p) == (0, 0):
                    filler_q.extend(proj_units(1))
                elif (j, p) == (1, 0):
                    flush()
                    filler_q.extend(proj_units(2))
                elif (j, p) == (2, 0):
                    flush()
                    filler_q.extend(proj_units(3))
                    in_last_window[0] = True
                    filler_q.extend(deferred_outproj)
                    deferred_outproj = []
                elif (j, p) == (3, 0):
                    flush()   # proj(3) must be complete before window 3
                advance_norms()
                f1, f2 = attn_pair(j, p)
                norm_q.append([0, j, f1, f2, False])
                if (j, p) == (0, 0):
                    proj_qk(0, "wq", 0, 1)
                    proj_qk(0, "wk", OC, 1)

            while norm_q:     # drain remaining norm stages
                advance_norms()
            flush()           # any leftover deferred outproj units
            for t in range(WT):
                outproj_unit(NWIN - 1, t, False)
                tick()
            tick(99)          # force out any deferred drains

    nc.compile()
    return nc


def make_in_maps(x, Wq, bq, Wk, bk, Wv, bv, Wo):
    BF = ml_dtypes.bfloat16
    # tri[p, f] = 1 where f >= p (keep key p for query f within a diag block)
    tri = np.triu(np.ones((P, P), dtype=np.float32)).astype(BF)
    in_maps = []
    for c in range(8):
        b, g = c // 2, c % 2
        sl = slice(g * DC, (g + 1) * DC)
        def warr(w):
            return np.ascontiguousarray(
                w.reshape(-1, P, w.shape[1]).transpose(1, 0, 2)).astype(BF)
        bias3 = np.zeros((P, 4 * OC), np.float32)
        bias3[:, 0:OC] = bq[sl].reshape(OC, P).T
        bias3[:, OC:2 * OC] = bk[sl].reshape(OC, P).T
        bias3[0:HD, 2 * OC:2 * OC + NHC] = bv[sl].reshape(NHC, HD).T
        xtb = np.ascontiguousarray(
            x[b].T.reshape(FC, P, -1).transpose(1, 0, 2)).astype(BF)
        in_maps.append({
            "xt": xtb,
            "wq": warr(Wq[:, sl]),
            "wk": warr(Wk[:, sl]),
            "wv": warr(Wv[:, sl]),
            "wo": warr(Wo[sl, :]),
            "bias3": np.ascontiguousarray(bias3.astype(np.float32)),
            "tri": tri,
        })
    return in_maps


_NC_CACHE = {}


def kernel(x, Wq, bq, Wk, bk, Wv, bv, Wo, bo):
    x = np.asarray(x, dtype=np.float32)
    args = [np.asarray(a, dtype=np.float32)
            for a in (Wq, bq, Wk, bk, Wv, bv, Wo, bo)]
    Wq, bq, Wk, bk, Wv, bv, Wo, bo = args
    key = ("nc", x.shape[1], bool(np.any(bv)))
    if key not in _NC_CACHE:
        _NC_CACHE[key] = build_nc(S=x.shape[1], num_devices=8,
                                  with_bv=bool(np.any(bv)))
    nc = _NC_CACHE[key]
    in_maps = make_in_maps(x, Wq, bq, Wk, bk, Wv, bv, Wo)
    res = run_bass_kernel_spmd(nc, in_maps, core_ids=list(range(8)))
    B = x.shape[0]
    out = np.empty_like(x)
    for b in range(B):
        out[b] = (res.results[2 * b]["out"].astype(np.float32)
                  + res.results[2 * b + 1]["out"].astype(np.float32) + bo)
    return out



# revision 35
# speedup vs baseline: 1.1446x; 1.1446x over previous
"""Causal multi-head attention layer (train forward) on 8 Trainium2 NeuronCores.

Sharding: batch (4) x head-group (2 of 8 heads each) -> 8 cores.
Per core (batch b, head group g): project Q^T/K^T [512,S] and V [S,512] from
x_b in bf16 (fp32 PSUM accum), run causal attention head-pair-packed on the PE
array (row-tiled 64x128 score matmuls run both heads concurrently on the
row-split PE tiles), softmax rowsums ride a ones column on V (ctx matmul
M=65), reciprocal_approx_fast on DVE (partition-base 0 only!), and the
output projection with this core's Wo row block (bf16 out partials summed on
host). Per-pair staged normalization: h0 ctx stages at partitions 0-63 and
h1 at partitions 64-127 (cross-base PSUM copy), the per-head reciprocal rows
are DMA-broadcast into the matching partition halves, and the normalization
multiplies write ctx_w in place for BOTH halves -- no SBUF->SBUF DMA
roundtrip for the upper half. Triangle masking runs on GpSimd (SBUF-only
engine). Projection/output-projection matmuls are interleaved into the
exp-bound attention stream as deficit-paced fillers so the PE keeps busy
while ACT works through the exp stream; PE-output drains are emitted 2
work-units late and alternate DVE/ACT so neither in-order queue head-blocks.
"""
import numpy as np
import ml_dtypes

import concourse.bass as bass
import concourse.tile as tile
from concourse import bacc, mybir
from concourse.bass_utils import run_bass_kernel_spmd

F32 = mybir.dt.float32
BF16 = mybir.dt.bfloat16
AF = mybir.ActivationFunctionType
ALU = mybir.AluOpType

P = 128
D = 1024          # model dim
DC = 512          # per-core head dims (8 heads x 64)
HD = 64
NHC = 8           # heads per core
NPAIR = 4         # head pairs per core
FC = D // P       # 8 feature chunks
OC = DC // P      # 4 outdim chunks (= head pairs)
W = 512           # query window (fp32 PSUM bank)
WT = W // P       # token chunks per window
SCALE = 1.0 / 32.0  # 1/sqrt(D)

PROJ_COST = 3413.0   # ns: one 16-matmul projection group
OUTP_COST = 1707.0   # ns: one 8-matmul outproj token group


def build_nc(S=2048, num_devices=8, with_bv=False):
    NWIN = S // W

    nc = bacc.Bacc("TRN2", target_bir_lowering=False, debug=False,
                   num_devices=num_devices)
    xt = nc.dram_tensor("xt", [P, FC, S], BF16, kind="ExternalInput").ap()
    wq = nc.dram_tensor("wq", [P, FC, DC], BF16, kind="ExternalInput").ap()
    wk = nc.dram_tensor("wk", [P, FC, DC], BF16, kind="ExternalInput").ap()
    wv = nc.dram_tensor("wv", [P, FC, DC], BF16, kind="ExternalInput").ap()
    wo = nc.dram_tensor("wo", [P, OC, D], BF16, kind="ExternalInput").ap()
    bias3 = nc.dram_tensor("bias3", [P, 4 * OC], F32,
                           kind="ExternalInput").ap()
    tri = nc.dram_tensor("tri", [P, P], BF16, kind="ExternalInput").ap()
    out = nc.dram_tensor("out", [S, D], BF16,
                         kind="ExternalOutput").ap()

    with tile.TileContext(nc) as tc:
        with tc.tile_pool(name="const", bufs=1) as cst, \
             tc.tile_pool(name="stage", bufs=3) as stg, \
             tc.tile_pool(name="pt", bufs=5) as ptp, \
             tc.tile_pool(name="small", bufs=2) as sml, \
             tc.tile_pool(name="stgp", bufs=2) as stgp, \
             tc.tile_pool(name="psA", bufs=1, space="PSUM") as psA, \
             tc.tile_pool(name="psC", bufs=1, space="PSUM") as psC:

            mm_ctr = [0]

            def mm_tile():
                i = mm_ctr[0]
                mm_ctr[0] += 1
                return psA.tile([P, 1024], F32, tag=f"s{i % 3}",
                                name=f"mm_s{i % 3}")

            eng_ctr = [0]

            def nxt():
                eng_ctr[0] += 1
                return eng_ctr[0]

            def _copy(out_, in_, dve_only=False):
                if dve_only or nxt() % 2 == 0:
                    nc.vector.tensor_copy(out_, in_)
                else:
                    nc.scalar.copy(out_, in_)

            # --- constants (already bf16/pre-arranged from host) ---
            tri_bf = cst.tile([P, P], BF16, tag="tri")
            b3_sb = cst.tile([P, 4 * OC], F32, tag="bias3")
            bv_sb = b3_sb[0:HD, 2 * OC:2 * OC + NHC]
            w_sbs = {}
            for name in ("wq", "wk", "wv"):
                w_sbs[name] = cst.tile([P, FC, DC], BF16, tag=name, name=name)
            wo_sb = cst.tile([P, OC, D], BF16, tag="wo")

            # --- per-window tiles ---
            xT_w, qT_w, kT_w, v_w, ctx_w = [], [], [], [], []
            for j in range(NWIN):
                xT_w.append(cst.tile([P, FC, W], BF16, tag=f"xT{j}",
                                     name=f"xT{j}"))
                qT_w.append(cst.tile([P, OC, W], BF16, tag=f"qT{j}",
                                     name=f"qT{j}"))
                kT_w.append(cst.tile([P, OC, W], BF16, tag=f"kT{j}",
                                     name=f"kT{j}"))
                v_w.append(cst.tile([P, WT, NHC, HD + 1], BF16, tag=f"v{j}",
                                    name=f"v{j}"))
                ctx_w.append(cst.tile([P, NPAIR, W], BF16, tag=f"ctx{j}",
                                      name=f"ctx{j}"))
                nc.vector.memset(v_w[j][:, :, :, HD:HD + 1], 1.0)

            # --- startup DMAs; one shared DMA engine, so order by need.
            # Gating inputs (xT0/wq/wk first halves) are chunked per feature
            # chunk so the first projections' fc-accumulation tracks DMA
            # progress. All later bulk prefetch goes on the scalar ring so
            # the sync/gpsimd rings stay free for small latency-critical
            # DMAs (bc broadcasts, shw, out chunks). ---
            HC = DC // 2
            # window-0 scores need x0 + wq/wk first halves (2MB): strict
            # consumption-order round-robin over the three DMA rings lands
            # the set at the ~30us 3-ring bandwidth floor instead of the
            # unbalanced ~36us. b3 leads (first drain needs it).
            nc.gpsimd.dma_start(b3_sb[:], bias3[:])
            rings = [nc.sync, nc.scalar, nc.gpsimd]
            k = 0
            for fc in range(FC):
                rings[k % 3].dma_start(xT_w[0][:, fc, :], xt[:, fc, 0:W])
                k += 1
                rings[k % 3].dma_start(w_sbs["wq"][:, fc, 0:HC],
                                       wq[:, fc, 0:HC])
                k += 1
                rings[k % 3].dma_start(w_sbs["wk"][:, fc, 0:HC],
                                       wk[:, fc, 0:HC])
                k += 1
            nc.gpsimd.dma_start(tri_bf[:], tri[:])
            for fc in range(FC):
                rings[k % 3].dma_start(w_sbs["wv"][:, fc, :], wv[:, fc, :])
                k += 1
            nc.scalar.dma_start(w_sbs["wq"][:, :, HC:DC], wq[:, :, HC:DC])
            nc.scalar.dma_start(w_sbs["wk"][:, :, HC:DC], wk[:, :, HC:DC])
            for jj in range(1, NWIN):
                nc.scalar.dma_start(xT_w[jj][:], xt[:, :, jj * W:(jj + 1) * W])
            nc.scalar.dma_start(wo_sb[:], wo[:])

            # --- projection / output-projection units (PE filler work) ---
            def proj_qk(j, wname, b_off, og):
                dst = qT_w[j] if wname == "wq" else kT_w[j]
                w_sb = w_sbs[wname]
                ps = mm_tile()
                for half in range(2):
                    oc = og * 2 + half
                    for fc in range(FC):
                        nc.tensor.matmul(
                            ps[:, half * W:(half + 1) * W],
                            w_sb[:, fc, oc * P:(oc + 1) * P],
                            xT_w[j][:, fc, :],
                            start=(fc == 0), stop=(fc == FC - 1))
                    hv = ps[:, half * W:(half + 1) * W]
                    b_col = b3_sb[:, b_off + oc:b_off + oc + 1]

                    def drain(hv=hv, b_col=b_col, oc=oc):
                        if nxt() % 2 == 0:
                            nc.vector.tensor_scalar(dst[:, oc, :], hv, b_col,
                                                    None, ALU.add)
                        else:
                            nc.scalar.activation(dst[:, oc, :], hv,
                                                 AF.Identity, bias=b_col)
                    if j == 0:
                        drain()     # window-0 ramp: attention waits on these
                    else:
                        defer(2, drain)

            def proj_v(j, tg):
                ps = mm_tile()
                for half in range(2):
                    t = tg * 2 + half
                    for fc in range(FC):
                        nc.tensor.matmul(
                            ps[:, half * W:(half + 1) * W],
                            xT_w[j][:, fc, t * P:(t + 1) * P],
                            w_sbs["wv"][:, fc, :],
                            start=(fc == 0), stop=(fc == FC - 1))
                dv = ps.rearrange("p (t h n) -> p t h n", t=2, h=NHC)

                def drain():
                    _copy(v_w[j][:, tg * 2:tg * 2 + 2, :, 0:HD], dv)
                if j == 0:
                    drain()
                else:
                    defer(2, drain)

            def outproj_unit(j, t, dve_drain):
                tokc = j * WT + t
                ps = mm_tile()
                # pr-major-first, last pair deferred to the end of BOTH psum
                # halves: the first six matmuls only need pairs 0-2, so at
                # the kernel tail they overlap the final pair's norm chain
                for nb in range(2):
                    for pr in range(NPAIR - 1):
                        nc.tensor.matmul(
                            ps[:, nb * W:(nb + 1) * W],
                            ctx_w[j][:, pr, t * P:(t + 1) * P],
                            wo_sb[:, pr, nb * 512:(nb + 1) * 512],
                            start=(pr == 0), stop=False)
                for nb in range(2):
                    nc.tensor.matmul(
                        ps[:, nb * W:(nb + 1) * W],
                        ctx_w[j][:, NPAIR - 1, t * P:(t + 1) * P],
                        wo_sb[:, NPAIR - 1, nb * 512:(nb + 1) * 512],
                        start=False, stop=True)
                def drain():
                    ost = stg.tile([P, D], BF16, tag="ostage", name="ost")
                    _copy(ost[:], ps[:], dve_only=dve_drain)
                    nc.sync.dma_start(out[tokc * P:(tokc + 1) * P, :], ost[:])
                defer(2, drain)

            # --- deferred drains: a drain emitted right after its producer
            # matmuls reaches the in-order DVE/ACT queue head while the PE
            # is still several units behind, and head-blocks the queue.
            # Defer by 2 PE work-units (< the 3-deep psum rotation, so the
            # drain is always emitted before the strip's next writer). ---
            defer_q = []

            def defer(delay, fn):
                defer_q.append([delay, fn])

            def tick(n=1):
                for it in defer_q[:]:
                    it[0] -= n
                    if it[0] <= 0:
                        defer_q.remove(it)
                        it[1]()

            # --- filler scheduler: PE work to hide ACT(exp) latency ---
            filler_q = []
            debt = [0.0]

            def pump(add_ns):
                debt[0] += add_ns
                while filler_q and debt[0] >= filler_q[0][0]:
                    cost, fn = filler_q.pop(0)
                    fn()
                    tick()
                    debt[0] -= cost
                if not filler_q and debt[0] > 4000.0:
                    debt[0] = 4000.0

            def flush():
                while filler_q:
                    _, fn = filler_q.pop(0)
                    fn()
                    tick()
                debt[0] = 0.0

            # --- attention for one (window, head-pair); returns norm closure ---
            def attn_pair(j, p):
                skc_hi = WT * (j + 1)
                last_win = (j == NWIN - 1)
                ctx0 = psC.tile([P, W], F32, tag="c0", name="ctx0")
                ctx1 = psC.tile([P, W], F32, tag="c1", name="ctx1")
                for skc in range(skc_hi):
                    jk, tk = divmod(skc, WT)
                    rel = skc * P - j * W
                    vs = max(rel, 0)
                    n = W - vs
                    sp = mm_tile()
                    spv = sp.rearrange("p (h n) -> p h n", h=2)
                    nc.tensor.matmul(sp[:, vs:W],
                                     kT_w[jk][0:HD, p, tk * P:(tk + 1) * P],
                                     qT_w[j][0:HD, p, vs:W],
                                     start=True, stop=True)
                    nc.tensor.matmul(sp[:, W + vs:2 * W],
                                     kT_w[jk][HD:P, p, tk * P:(tk + 1) * P],
                                     qT_w[j][HD:P, p, vs:W],
                                     start=True, stop=True)
                    pt = ptp.tile([P, 1024], BF16, tag="pt", name="pt")
                    ptv = pt.rearrange("p (h n) -> p h n", h=2)
                    nc.scalar.activation(ptv[:, :, vs:W], spv[:, :, vs:W],
                                         AF.Exp, scale=SCALE)
                    if rel >= 0:
                        nc.gpsimd.tensor_tensor(
                            ptv[:, :, rel:rel + P], ptv[:, :, rel:rel + P],
                            tri_bf[:, None, :].to_broadcast([P, 2, P]),
                            ALU.mult)
                    st0 = (skc == 0)
                    sp0 = (skc == skc_hi - 1)
                    nc.tensor.matmul(ctx0[0:HD + 1, vs:W],
                                     v_w[jk][:, tk, 2 * p, :],
                                     ptv[:, 0, vs:W], start=st0, stop=sp0)
                    nc.tensor.matmul(ctx1[0:HD + 1, vs:W],
                                     v_w[jk][:, tk, 2 * p + 1, :],
                                     ptv[:, 1, vs:W], start=st0, stop=sp0)
                    # ACT time minus PE time for this chunk -> filler budget
                    tick()
                    pump((2.0 * n + 352.0) / 1.2 - (3.0 * n / 2.4 + 120.0))

                # rowsums (ones column) + unnormalized ctx drains; h1 ctx
                # stages directly at partitions 64-127 (cross-base copy) so
                # the normalization can write ctx_w[64:128] in place -- no
                # shw DMA roundtrip at all.
                stg2 = stgp.tile([P, 2, W], BF16, tag=f"sg{p % 2}",
                                 name="stg2")
                rws = []
                for h, cp in ((0, ctx0), (1, ctx1)):
                    rw = sml.tile([1, W], F32, tag=f"rw{(2 * p + h) % 4}",
                                  name="rw")
                    rws.append(rw)
                    nc.vector.tensor_copy(rw[:], cp[HD:HD + 1, :])
                    dst2 = (stg2[0:HD, 0, :] if h == 0
                            else stg2[HD:P, 1, :])
                    _copy(dst2, cp[0:HD, :], dve_only=(last_win or h == 1))

                bcs = []

                def stage1():
                    # reciprocal + bf16 cast + broadcast-DMA issue, one pair
                    # boundary after the rowsums landed. bc halves land at
                    # the partitions their head's ctx occupies.
                    bc = sml.tile([P, W], BF16, tag=f"bc{p % 2}", name="bc")
                    bcs.append(bc)
                    for h in range(2):
                        i = 2 * p + h
                        rcf = sml.tile([1, W], F32, tag=f"rf{i % 2}",
                                       name="rcf")
                        nc.vector.reciprocal_approx_fast(rcf[:], rws[h][:])
                        rc1 = sml.tile([1, W], BF16, tag=f"rc{i % 2}",
                                       name="rc1")
                        nc.vector.tensor_copy(rc1[:], rcf[:])
                        if j == NWIN - 1 and p >= 2:
                            ring = nc.scalar if h == 0 else nc.sync
                        else:
                            ring = nc.gpsimd
                        ring.dma_start(
                            bc[h * HD:(h + 1) * HD, :],
                            rc1[0:1, None, :].to_broadcast([1, HD, W]))

                def stage2():
                    # normalization multiplies; two pair boundaries after the
                    # rowsums, so the bc broadcasts are ready
                    bc = bcs[0]
                    for h in range(2):
                        i = 2 * p + h
                        dst = (ctx_w[j][0:HD, p, :] if h == 0
                               else ctx_w[j][HD:P, p, :])
                        src2 = (stg2[0:HD, 0, :] if h == 0
                                else stg2[HD:P, 1, :])
                        nc.vector.tensor_tensor(
                            dst, src2, bc[h * HD:(h + 1) * HD, :], ALU.mult)
                        if with_bv:
                            nc.vector.tensor_scalar(
                                dst, dst, bv_sb[:, i:i + 1], None, ALU.add)
                return stage1, stage2

            # --- main schedule ---
            proj_qk(0, "wq", 0, 0)
            proj_qk(0, "wk", OC, 0)
            proj_v(0, 0)
            proj_v(0, 1)

            deferred_outproj = []
            norm_q = []       # [age, pj, s1, s2, s1_done]
            norms_done = {}
            in_last_window = [False]

            def advance_norms():
                for ent in norm_q[:]:
                    ent[0] += 1
                    age, pj, f1, f2, s1_done = ent
                    if age >= 1 and not s1_done:
                        f1()
                        ent[4] = True
                    # late windows have long pairs: 2 boundaries of slack
                    # suffice for the bc broadcast, and the earlier finish
                    # lets outproj(2) interleave instead of piling at the tail
                    if age >= (3 if pj <= 1 else 2):
                        f2()
                        norm_q.remove(ent)
                        norms_done[pj] = norms_done.get(pj, 0) + 1
                        if norms_done[pj] == NPAIR and pj < NWIN - 1:
                            units = [(OUTP_COST,
                                      lambda jj=pj, tt=t:
                                      outproj_unit(jj, tt, True))
                                     for t in range(WT)]
                            if in_last_window[0]:
                                filler_q.extend(units)
                            else:
                                deferred_outproj.extend(units)

            def proj_units(jn):
                return [
                    (PROJ_COST, lambda: proj_qk(jn, "wq", 0, 0)),
                    (PROJ_COST, lambda: proj_qk(jn, "wk", OC, 0)),
                    (PROJ_COST, lambda: proj_v(jn, 0)),
                    (PROJ_COST, lambda: proj_v(jn, 1)),
                    (PROJ_COST, lambda: proj_qk(jn, "wq", 0, 1)),
                    (PROJ_COST, lambda: proj_qk(jn, "wk", OC, 1)),
                ]

            # windows 0,1 sequential; windows 2,3 pair-interleaved so
            # window 3's exp load spreads into the PE-rich window-2 phase.
            # proj(3) runs as fillers inside the mixed phase (needed only
            # by the first (3,*) pair).
            sched = [(0, p) for p in range(NPAIR)]
            sched += [(1, p) for p in range(NPAIR)]
            sched += [(2, 0), (2, 1), (2, 2), (3, 0),
                      (2, 3), (3, 1), (3, 2), (3, 3)]

            for (j, p) in sched:
                if (j, p) == (0, 0):
                    filler_q.extend(proj_units(1))
                elif (j, p) == (1, 0):
                    flush()
                    filler_q.extend(proj_units(2))
                elif (j, p) == (2, 0):
                    flush()
                    filler_q.extend(proj_units(3))
                    in_last_window[0] = True
                    filler_q.extend(deferred_outproj)
                    deferred_outproj = []
                elif (j, p) == (3, 0):
                    flush()   # proj(3) must be complete before window 3
                advance_norms()
                f1, f2 = attn_pair(j, p)
                norm_q.append([0, j, f1, f2, False])
                if (j, p) == (0, 0):
                    proj_qk(0, "wq", 0, 1)
                    proj_qk(0, "wk", OC, 1)

            while norm_q:     # drain remaining norm stages
                advance_norms()
            flush()           # any leftover deferred outproj units
            for t in range(WT):
                outproj_unit(NWIN - 1, t, False)
                tick()
            tick(99)          # force out any deferred drains

    nc.compile()
    return nc


def make_in_maps(x, Wq, bq, Wk, bk, Wv, bv, Wo):
    BF = ml_dtypes.bfloat16
    # tri[p, f] = 1 where f >= p (keep key p for query f within a diag block)
    tri = np.triu(np.ones((P, P), dtype=np.float32)).astype(BF)
    in_maps = []
    for c in range(8):
        b, g = c // 2, c % 2
        sl = slice(g * DC, (g + 1) * DC)
        def warr(w):
            return np.ascontiguousarray(
                w.reshape(-1, P, w.shape[1]).transpose(1, 0, 2)).astype(BF)
        bias3 = np.zeros((P, 4 * OC), np.float32)
        bias3[:, 0:OC] = bq[sl].reshape(OC, P).T
        bias3[:, OC:2 * OC] = bk[sl].reshape(OC, P).T
        bias3[0:HD, 2 * OC:2 * OC + NHC] = bv[sl].reshape(NHC, HD).T
        xtb = np.ascontiguousarray(
            x[b].T.reshape(FC, P, -1).transpose(1, 0, 2)).astype(BF)
        in_maps.append({
            "xt": xtb,
            "wq": warr(Wq[:, sl]),
            "wk": warr(Wk[:, sl]),
            "wv": warr(Wv[:, sl]),
            "wo": warr(Wo[sl, :]),
            "bias3": np.ascontiguousarray(bias3.astype(np.float32)),
            "tri": tri,
        })
    return in_maps


_NC_CACHE = {}


def kernel(x, Wq, bq, Wk, bk, Wv, bv, Wo, bo):
    x = np.asarray(x, dtype=np.float32)
    args = [np.asarray(a, dtype=np.float32)
            for a in (Wq, bq, Wk, bk, Wv, bv, Wo, bo)]
    Wq, bq, Wk, bk, Wv, bv, Wo, bo = args
    key = ("nc", x.shape[1], bool(np.any(bv)))
    if key not in _NC_CACHE:
        _NC_CACHE[key] = build_nc(S=x.shape[1], num_devices=8,
                                  with_bv=bool(np.any(bv)))
    nc = _NC_CACHE[key]
    in_maps = make_in_maps(x, Wq, bq, Wk, bk, Wv, bv, Wo)
    res = run_bass_kernel_spmd(nc, in_maps, core_ids=list(range(8)))
    B = x.shape[0]
    out = np.empty_like(x)
    for b in range(B):
        out[b] = (res.results[2 * b]["out"].astype(np.float32)
                  + res.results[2 * b + 1]["out"].astype(np.float32) + bo)
    return out

